# revision 12
# baseline (speedup 1.0000x reference)
"""Trainium2 Bass kernel for the BaseHeads pairwise-tanh head.

Computes, for x:(B,S,H)=(2,128,768), R=4 heads:
    s = x @ w_src.T + b_src   -> (B,S,R,H)
    t = x @ w_tgt.T + b_tgt   -> (B,S,R,H)
    out[b,r,i,j] = sum_h tanh(s[b,i,r,h] + t[b,j,r,h]) * w_out[h]

Sharding: one (b, r) pair per NeuronCore (B*R == 8 == n_cores), no
collectives.  Each core gets its own pre-transposed weight slices and
x[b]^T (host-prepped, bf16) and returns logits^T (j, i) for its pair.

Per-core dataflow (all static/unrolled, Tile framework):
  PE  : 12x (6 accumulating 128x128 matmuls)  -> s_T/t_T (h on partitions)
  DVE : 768x tensor_scalar_add (t_T chunk + per-partition s column)
  ACT : in-place big-tile Tanh (+ per-partition combined bias)
  PE  : 768x (LDW + N=1 matmul): lhsT = tanh tile (K=h, M=j), rhs = w_out
        chunk (K=h, 1); each column accumulates in its own PSUM bank
  DVE : batched strided PSUM->SBUF drains; one DMA out (64KB)

This walrus build allows AT MOST ONE sync-wait per engine instruction, so
the dataflow is arranged so every instruction has cross-engine deps from
at most one other engine (waits on the same semaphore merge):
  - tanh reads only DVE-written tiles (adds output + DVE-copied bias);
  - the slot-reuse WAR vs PE is carried by the first tensor_scalar_add;
  - PE pre-observes DVE/ACT progress once per block via two dummy
    load_weights on single-writer flag tiles (a DVE memset flag and the
    last tanh's accum_out), so the real Ldweights need no waits.
"""

import sys

if "/opt/trn_rl_repo" not in sys.path:
    sys.path.insert(0, "/opt/trn_rl_repo")

import ml_dtypes
import numpy as np

B, S, H, R = 2, 128, 768, 4
KC = H // 128  # 6 h-chunks
N_CORES = 8
I_BLK = 32  # i's per A-tile macro block
N_BLKS = S // I_BLK
DRAIN_W = 4  # columns per PSUM drain batch (each column in its own bank)

BF16 = ml_dtypes.bfloat16

_PROGRAM_CACHE = {}
LAST_RESULTS = None  # BassKernelResults of the most recent run (for test.py)


def _build_program(split=True):
    import concourse.bass as bass
    import concourse.mybir as mybir
    from concourse.tile import TileContext

    f32 = mybir.dt.float32
    bf16 = mybir.dt.bfloat16

    nc = bass.Bass()

    # Inputs (per-core, host pre-transposed, bf16 except biases).
    # xt  : (128, 768)  [p, kc*128+i]  = x[b].T chunk layout
    # ws  : (128, 4608) [p, m*768+kc*128+j] = w_src_r.T slab layout
    # wt  : (128, 4608) same for w_tgt_r.T
    # bc  : (128, 6)    [p, m] = (b_src+b_tgt)[r*768+m*128+p]  (f32)
    # wo  : (128, 6)    [p, c] = w_out[c*128+p]
    xt_d = nc.dram_tensor("xt", [128, H], bf16, kind="ExternalInput")
    ws_d = nc.dram_tensor("ws", [128, KC * H], bf16, kind="ExternalInput")
    wt_d = nc.dram_tensor("wt", [128, KC * H], bf16, kind="ExternalInput")
    bc_d = nc.dram_tensor("bc", [128, KC], f32, kind="ExternalInput")
    wo_d = nc.dram_tensor("wo", [128, KC], bf16, kind="ExternalInput")
    out_d = nc.dram_tensor("outT", [S, S], f32, kind="ExternalOutput")

    Tanh = mybir.ActivationFunctionType.Tanh

    with TileContext(nc) as tc:
        with (
            tc.tile_pool(name="const", bufs=1) as const_pool,
            tc.tile_pool(name="wpool", bufs=1) as w_pool,
            tc.tile_pool(name="apool", bufs=2) as a_pool,
            tc.tile_pool(name="flags", bufs=2) as flag_pool,
        ):
            x_t = const_pool.tile([128, H], bf16, tag="xt")
            bc_t = const_pool.tile([128, KC], f32, tag="bc")
            wo_t = const_pool.tile([128, KC], bf16, tag="wo")
            bc_v = const_pool.tile([128, KC], f32, tag="bcv")
            out_sb = const_pool.tile([S, S], f32, tag="osb")
            nc.sync.dma_start(out=x_t, in_=xt_d[:, :])
            nc.sync.dma_start(out=bc_t, in_=bc_d[:, :])
            nc.sync.dma_start(out=wo_t, in_=wo_d[:, :])
            # DVE-local copy of the bias so the tanh's only cross-engine
            # dep proc is DVE.
            nc.vector.tensor_copy(bc_v, bc_t)

            s_T = [const_pool.tile([128, 128], f32, tag=f"s{m}", name=f"s_T{m}") for m in range(KC)]
            t_T = [const_pool.tile([128, 128], bf16, tag=f"t{m}", name=f"t_T{m}") for m in range(KC)]

            # ---- projections: s_T[m][h_local, i], t_T[m][h_local, j] ----
            with tc.tile_pool(name="psproj", bufs=2, space="PSUM") as ps_proj:
                for m in range(KC):
                    for side in ("s", "t"):
                        wm = w_pool.tile([128, H], bf16, tag=f"w{side}{m}", name=f"w{side}{m}")
                        src = ws_d if side == "s" else wt_d
                        nc.sync.dma_start(out=wm, in_=src[:, m * H : (m + 1) * H])
                        ps = ps_proj.tile([128, 128], f32, tag="pp", name=f"pp_{side}{m}")
                        for kc in range(KC):
                            nc.tensor.matmul(
                                ps,
                                wm[:, kc * 128 : (kc + 1) * 128],
                                x_t[:, kc * 128 : (kc + 1) * 128],
                                start=(kc == 0),
                                stop=(kc == KC - 1),
                            )
                        dst = s_T[m] if side == "s" else t_T[m]
                        nc.vector.tensor_copy(dst, ps)

            # ---- pairwise tanh + weighted reduction ----
            with tc.tile_pool(name="psout", bufs=2, space="PSUM") as ps_out:
                for n in range(N_BLKS):
                    a_tiles = []
                    acc = flag_pool.tile([128, 1], f32, tag="acc", name=f"acc{n}")
                    flg = flag_pool.tile([128, 1], bf16, tag="flg", name=f"flg{n}")
                    for c in range(KC):
                        a = a_pool.tile([128, I_BLK * 128], bf16, tag=f"a{c}", name=f"a{n}_{c}")
                        for il in range(I_BLK):
                            i = n * I_BLK + il
                            nc.vector.tensor_scalar_add(
                                a[:, il * 128 : (il + 1) * 128],
                                t_T[c],
                                s_T[c][:, i : i + 1],
                            )
                        a_tiles.append(a)
                    # DVE flag: its tick dominates every add of this block.
                    nc.vector.memset(flg, 0.0)
                    for c in range(KC):
                        a = a_tiles[c]
                        if c == KC - 1:
                            nc.scalar.activation(
                                a, a, Tanh, bias=bc_v[:, c : c + 1], scale=1.0,
                                accum_out=acc,
                            )
                        else:
                            nc.scalar.activation(
                                a, a, Tanh, bias=bc_v[:, c : c + 1], scale=1.0
                            )
                    # PE observers: one DVE wait (flag) + one ACT wait (acc
                    # written by the last tanh) so the real Ldweights below
                    # carry no waits.
                    nc.tensor.ldweights(flg[:, :])
                    nc.tensor.ldweights(acc.bitcast(bf16))
                    for batch in range(I_BLK // DRAIN_W):
                        pt = ps_out.tile([128, DRAIN_W, 512], f32, tag="lp", name=f"lp{n}_{batch}")
                        for q in range(DRAIN_W):
                            il = batch * DRAIN_W + q
                            for c in range(KC):
                                nc.tensor.matmul(
                                    pt[:, q, 0:1],
                                    a_tiles[c][:, il * 128 : (il + 1) * 128],
                                    wo_t[:, c : c + 1],
                                    start=(c == 0),
                                    stop=(c == KC - 1),
                                )
                        i0 = n * I_BLK + batch * DRAIN_W
                        nc.vector.tensor_copy(out_sb[:, i0 : i0 + DRAIN_W], pt[:, :, 0])

            nc.sync.dma_start(out=out_d[:, :], in_=out_sb)

    if split:
        _split_multi_waits(nc, mybir)
    return nc


def _split_multi_waits(nc, mybir):
    """This walrus build allows at most ONE sync-wait per instruction.
    Legalize by hoisting all but one wait onto same-engine NoOps placed
    immediately before the offending instruction (the engine executes its
    queue in order, so waiting on the NoOps first is equivalent)."""
    k = 0
    for func in nc.m.functions:
        for blk in func.blocks:
            insts = list(blk.instructions)
            out = []
            changed = False
            for inst in insts:
                si = inst.sync_info
                waits = list(si.on_wait) if si is not None and si.on_wait else []
                if len(waits) > 1:
                    changed = True
                    for w in waits[:-1]:
                        nop = mybir.InstNoOp(
                            name=f"WSPLIT-{k}",
                            engine=inst.engine,
                            sync_info=mybir.SyncInfo(on_wait=[w], on_update=[]),
                            ins=[],
                            outs=[],
                        )
                        k += 1
                        out.append(nop)
                    si.on_wait = [waits[-1]]
                out.append(inst)
            if changed:
                blk.instructions = out


def _prep_inputs(input_hidden_state, w_src, b_src, w_tgt, b_tgt, w_out):
    """Build the 8 per-core input dicts (host-side transpose/cast)."""
    x = np.asarray(input_hidden_state, dtype=np.float32)
    w_src = np.asarray(w_src, dtype=np.float32)
    w_tgt = np.asarray(w_tgt, dtype=np.float32)
    b_sum = np.asarray(b_src, dtype=np.float32) + np.asarray(b_tgt, dtype=np.float32)
    w_out = np.asarray(w_out, dtype=np.float32)

    wo_tile = np.ascontiguousarray(w_out.reshape(KC, 128).T).astype(BF16)

    in_maps = []
    for core in range(N_CORES):
        b, r = divmod(core, R)
        # xT chunk layout: xt[p, kc*128+i] = x[b][i, kc*128+p]
        xT = x[b].T  # (H, S)
        xt = np.ascontiguousarray(
            xT.reshape(KC, 128, S).transpose(1, 0, 2).reshape(128, H)
        ).astype(BF16)

        # ws[p, m*768 + kc*128 + j] = wT[kc*128+p, m*128+j],  wT = w_r.T
        wT_s = w_src[r * H : (r + 1) * H, :].T.reshape(KC, 128, KC, 128)
        ws = np.ascontiguousarray(
            wT_s.transpose(1, 2, 0, 3).reshape(128, KC * H)
        ).astype(BF16)
        wT_t = w_tgt[r * H : (r + 1) * H, :].T.reshape(KC, 128, KC, 128)
        wt = np.ascontiguousarray(
            wT_t.transpose(1, 2, 0, 3).reshape(128, KC * H)
        ).astype(BF16)

        bc = np.ascontiguousarray(
            b_sum[r * H : (r + 1) * H].reshape(KC, 128).T
        ).astype(np.float32)

        in_maps.append({"xt": xt, "ws": ws, "wt": wt, "bc": bc, "wo": wo_tile})
    return in_maps


def kernel(input_hidden_state, w_src, b_src, w_tgt, b_tgt, w_out):
    global LAST_RESULTS
    from concourse.bass_utils import run_bass_kernel_spmd

    if "prog" not in _PROGRAM_CACHE:
        _PROGRAM_CACHE["prog"] = _build_program()
    nc = _PROGRAM_CACHE["prog"]

    in_maps = _prep_inputs(
        input_hidden_state, w_src, b_src, w_tgt, b_tgt, w_out
    )
    res = run_bass_kernel_spmd(nc, in_maps, core_ids=list(range(N_CORES)))
    LAST_RESULTS = res

    out = np.empty((B, R, S, S), dtype=np.float32)
    for core in range(N_CORES):
        b, r = divmod(core, R)
        out[b, r] = np.asarray(res.results[core]["outT"], dtype=np.float32).T
    return out


# revision 14
# speedup vs baseline: 1.0753x; 1.0753x over previous
"""Trainium2 Bass kernel for the BaseHeads pairwise-tanh head.

Computes, for x:(B,S,H)=(2,128,768), R=4 heads:
    s = x @ w_src.T + b_src   -> (B,S,R,H)
    t = x @ w_tgt.T + b_tgt   -> (B,S,R,H)
    out[b,r,i,j] = sum_h tanh(s[b,i,r,h] + t[b,j,r,h]) * w_out[h]

Sharding: one (b, r) pair per NeuronCore (B*R == 8 == n_cores), no
collectives.  Each core gets its own pre-transposed weight slices and
x[b]^T (host-prepped, bf16) and returns logits^T (j, i) for its pair.

Per-core dataflow (all static/unrolled, Tile framework):
  PE  : 12x (6 accumulating 128x128 matmuls)  -> s_T/t_T (h on partitions)
  DVE : 768x tensor_scalar_add (t_T chunk + per-partition s column)
  ACT : in-place big-tile Tanh (+ per-partition combined bias)
  PE  : 768x (LDW + N=1 matmul): lhsT = tanh tile (K=h, M=j), rhs = w_out
        chunk (K=h, 1); each column accumulates in its own PSUM bank
  DVE : batched strided PSUM->SBUF drains; one DMA out (64KB)

This walrus build allows AT MOST ONE sync-wait per engine instruction, so
the dataflow is arranged so every instruction has cross-engine deps from
at most one other engine (waits on the same semaphore merge):
  - tanh reads only DVE-written tiles (adds output + DVE-copied bias);
  - the slot-reuse WAR vs PE is carried by the first tensor_scalar_add;
  - PE pre-observes DVE/ACT progress once per block via two dummy
    load_weights on single-writer flag tiles (a DVE memset flag and the
    last tanh's accum_out), so the real Ldweights need no waits.
"""

import sys

if "/opt/trn_rl_repo" not in sys.path:
    sys.path.insert(0, "/opt/trn_rl_repo")

import ml_dtypes
import numpy as np

B, S, H, R = 2, 128, 768, 4
KC = H // 128  # 6 h-chunks
N_CORES = 8
I_BLK = 32  # i's per A-tile macro block
N_BLKS = S // I_BLK
DRAIN_W = 4  # columns per PSUM drain batch (each column in its own bank)

BF16 = ml_dtypes.bfloat16

_PROGRAM_CACHE = {}
LAST_RESULTS = None  # BassKernelResults of the most recent run (for test.py)


def _build_program(split=True):
    import concourse.bass as bass
    import concourse.mybir as mybir
    from concourse.tile import TileContext

    f32 = mybir.dt.float32
    bf16 = mybir.dt.bfloat16

    nc = bass.Bass()

    # Inputs (per-core, host pre-transposed, bf16 except biases).
    # xt  : (128, 768)  [p, kc*128+i]  = x[b].T chunk layout
    # ws  : (128, 4608) [p, m*768+kc*128+j] = w_src_r.T slab layout
    # wt  : (128, 4608) same for w_tgt_r.T
    # bc  : (128, 6)    [p, m] = (b_src+b_tgt)[r*768+m*128+p]  (f32)
    # wo  : (128, 6)    [p, c] = w_out[c*128+p]
    xt_d = nc.dram_tensor("xt", [128, H], bf16, kind="ExternalInput")
    ws_d = nc.dram_tensor("ws", [128, KC * H], bf16, kind="ExternalInput")
    wt_d = nc.dram_tensor("wt", [128, KC * H], bf16, kind="ExternalInput")
    bc_d = nc.dram_tensor("bc", [128, KC], f32, kind="ExternalInput")
    wo_d = nc.dram_tensor("wo", [128, KC], bf16, kind="ExternalInput")
    out_d = nc.dram_tensor("outT", [S * S // 512, 512], f32, kind="ExternalOutput")

    Tanh = mybir.ActivationFunctionType.Tanh

    with TileContext(nc) as tc:
        with (
            tc.tile_pool(name="const", bufs=1) as const_pool,
            tc.tile_pool(name="wpool", bufs=1) as w_pool,
            tc.tile_pool(name="apool", bufs=2) as a_pool,
        ):
            x_t = const_pool.tile([128, H], bf16, tag="xt")
            bc_t = const_pool.tile([128, KC], f32, tag="bc")
            wo_t = const_pool.tile([128, KC], bf16, tag="wo")
            bc_v = const_pool.tile([128, KC], f32, tag="bcv")
            out_sb = const_pool.tile([1, S * S], f32, tag="osb")
            nc.sync.dma_start(out=x_t, in_=xt_d[:, :])
            nc.sync.dma_start(out=bc_t, in_=bc_d[:, :])
            nc.sync.dma_start(out=wo_t, in_=wo_d[:, :])
            # DVE-local copy of the bias so the tanh's only cross-engine
            # dep proc is DVE.
            nc.vector.tensor_copy(bc_v, bc_t)

            s_T = [const_pool.tile([128, 128], bf16, tag=f"s{m}", name=f"s_T{m}") for m in range(KC)]
            t_T = [const_pool.tile([128, 128], bf16, tag=f"t{m}", name=f"t_T{m}") for m in range(KC)]

            # ---- projections: s_T[m][h_local, i], t_T[m][h_local, j] ----
            with tc.tile_pool(name="psproj", bufs=2, space="PSUM") as ps_proj:
                for m in range(KC):
                    for side in ("s", "t"):
                        wm = w_pool.tile([128, H], bf16, tag=f"w{side}{m}", name=f"w{side}{m}")
                        src = ws_d if side == "s" else wt_d
                        nc.sync.dma_start(out=wm, in_=src[:, m * H : (m + 1) * H])
                        ps = ps_proj.tile([128, 128], f32, tag="pp", name=f"pp_{side}{m}")
                        for kc in range(KC):
                            nc.tensor.matmul(
                                ps,
                                wm[:, kc * 128 : (kc + 1) * 128],
                                x_t[:, kc * 128 : (kc + 1) * 128],
                                start=(kc == 0),
                                stop=(kc == KC - 1),
                            )
                        dst = s_T[m] if side == "s" else t_T[m]
                        nc.vector.tensor_copy(dst, ps)

            # ---- pairwise tanh + weighted reduction ----
            # Per (block, chunk): one fat broadcast tensor_add builds the
            # (128, I_BLK, 128) tanh-argument tile, one in-place Tanh (with
            # the combined per-partition bias), then the reduction streams
            # the tanh tile as the MOVING matmul operand (N=512) against the
            # stationary w_out chunk column, accumulating (1, 512) rows of
            # logits (pair-major [i, j]) in PSUM.
            with tc.tile_pool(name="psout", bufs=4, space="PSUM") as ps_out:
                for n in range(N_BLKS):
                    a_tiles = []
                    for c in range(KC):
                        a = a_pool.tile([128, I_BLK, 128], bf16, tag=f"a{c}", name=f"a{n}_{c}")
                        sblk = s_T[c][:, n * I_BLK : (n + 1) * I_BLK]
                        nc.vector.tensor_add(
                            a[:, :, :],
                            sblk.unsqueeze(2).broadcast_to((128, I_BLK, 128)),
                            t_T[c].unsqueeze(1).broadcast_to((128, I_BLK, 128)),
                        )
                        nc.scalar.activation(
                            a[:, :, :], a[:, :, :], Tanh,
                            bias=bc_v[:, c : c + 1], scale=1.0,
                        )
                        a_tiles.append(a)
                    for g in range(I_BLK // 4):
                        ps = ps_out.tile([1, 512], f32, tag="lp", name=f"lp{n}_{g}")
                        for c in range(KC):
                            nc.tensor.matmul(
                                ps,
                                wo_t[:, c : c + 1],
                                a_tiles[c][:, g * 4 : (g + 1) * 4, :],
                                start=(c == 0),
                                stop=(c == KC - 1),
                            )
                        row = n * (I_BLK // 4) + g
                        nc.vector.tensor_copy(
                            out_sb[0:1, row * 512 : (row + 1) * 512], ps
                        )

            nc.sync.dma_start(out=out_d[:, :], in_=out_sb)

    if split:
        _split_multi_waits(nc, mybir)
    return nc


def _split_multi_waits(nc, mybir):
    """This walrus build allows at most ONE sync-wait per instruction.
    Legalize by hoisting all but one wait onto same-engine NoOps placed
    immediately before the offending instruction (the engine executes its
    queue in order, so waiting on the NoOps first is equivalent)."""
    k = 0
    for func in nc.m.functions:
        for blk in func.blocks:
            insts = list(blk.instructions)
            out = []
            changed = False
            for inst in insts:
                si = inst.sync_info
                waits = list(si.on_wait) if si is not None and si.on_wait else []
                if len(waits) > 1:
                    changed = True
                    for w in waits[:-1]:
                        nop = mybir.InstNoOp(
                            name=f"WSPLIT-{k}",
                            engine=inst.engine,
                            sync_info=mybir.SyncInfo(on_wait=[w], on_update=[]),
                            ins=[],
                            outs=[],
                        )
                        k += 1
                        out.append(nop)
                    si.on_wait = [waits[-1]]
                out.append(inst)
            if changed:
                blk.instructions = out


def _prep_inputs(input_hidden_state, w_src, b_src, w_tgt, b_tgt, w_out):
    """Build the 8 per-core input dicts (host-side transpose/cast)."""
    x = np.asarray(input_hidden_state, dtype=np.float32)
    w_src = np.asarray(w_src, dtype=np.float32)
    w_tgt = np.asarray(w_tgt, dtype=np.float32)
    b_sum = np.asarray(b_src, dtype=np.float32) + np.asarray(b_tgt, dtype=np.float32)
    w_out = np.asarray(w_out, dtype=np.float32)

    wo_tile = np.ascontiguousarray(w_out.reshape(KC, 128).T).astype(BF16)

    in_maps = []
    for core in range(N_CORES):
        b, r = divmod(core, R)
        # xT chunk layout: xt[p, kc*128+i] = x[b][i, kc*128+p]
        xT = x[b].T  # (H, S)
        xt = np.ascontiguousarray(
            xT.reshape(KC, 128, S).transpose(1, 0, 2).reshape(128, H)
        ).astype(BF16)

        # ws[p, m*768 + kc*128 + j] = wT[kc*128+p, m*128+j],  wT = w_r.T
        wT_s = w_src[r * H : (r + 1) * H, :].T.reshape(KC, 128, KC, 128)
        ws = np.ascontiguousarray(
            wT_s.transpose(1, 2, 0, 3).reshape(128, KC * H)
        ).astype(BF16)
        wT_t = w_tgt[r * H : (r + 1) * H, :].T.reshape(KC, 128, KC, 128)
        wt = np.ascontiguousarray(
            wT_t.transpose(1, 2, 0, 3).reshape(128, KC * H)
        ).astype(BF16)

        bc = np.ascontiguousarray(
            b_sum[r * H : (r + 1) * H].reshape(KC, 128).T
        ).astype(np.float32)

        in_maps.append({"xt": xt, "ws": ws, "wt": wt, "bc": bc, "wo": wo_tile})
    return in_maps


def kernel(input_hidden_state, w_src, b_src, w_tgt, b_tgt, w_out):
    global LAST_RESULTS
    from concourse.bass_utils import run_bass_kernel_spmd

    if "prog" not in _PROGRAM_CACHE:
        _PROGRAM_CACHE["prog"] = _build_program()
    nc = _PROGRAM_CACHE["prog"]

    in_maps = _prep_inputs(
        input_hidden_state, w_src, b_src, w_tgt, b_tgt, w_out
    )
    res = run_bass_kernel_spmd(nc, in_maps, core_ids=list(range(N_CORES)))
    LAST_RESULTS = res

    out = np.empty((B, R, S, S), dtype=np.float32)
    for core in range(N_CORES):
        b, r = divmod(core, R)
        out[b, r] = np.asarray(res.results[core]["outT"], dtype=np.float32).reshape(S, S)
    return out


# revision 15
# speedup vs baseline: 1.2105x; 1.1258x over previous
"""Trainium2 Bass kernel for the BaseHeads pairwise-tanh head.

Computes, for x:(B,S,H)=(2,128,768), R=4 heads:
    s = x @ w_src.T + b_src   -> (B,S,R,H)
    t = x @ w_tgt.T + b_tgt   -> (B,S,R,H)
    out[b,r,i,j] = sum_h tanh(s[b,i,r,h] + t[b,j,r,h]) * w_out[h]

Sharding: one (b, r) pair per NeuronCore (B*R == 8 == n_cores), no
collectives.  Each core gets its own pre-transposed weight slices and
x[b]^T (host-prepped, bf16) and returns logits^T (j, i) for its pair.

Per-core dataflow (all static/unrolled, Tile framework):
  PE  : 12x (6 accumulating 128x128 matmuls)  -> s_T/t_T (h on partitions)
  DVE : 768x tensor_scalar_add (t_T chunk + per-partition s column)
  ACT : in-place big-tile Tanh (+ per-partition combined bias)
  PE  : 768x (LDW + N=1 matmul): lhsT = tanh tile (K=h, M=j), rhs = w_out
        chunk (K=h, 1); each column accumulates in its own PSUM bank
  DVE : batched strided PSUM->SBUF drains; one DMA out (64KB)

This walrus build allows AT MOST ONE sync-wait per engine instruction, so
the dataflow is arranged so every instruction has cross-engine deps from
at most one other engine (waits on the same semaphore merge):
  - tanh reads only DVE-written tiles (adds output + DVE-copied bias);
  - the slot-reuse WAR vs PE is carried by the first tensor_scalar_add;
  - PE pre-observes DVE/ACT progress once per block via two dummy
    load_weights on single-writer flag tiles (a DVE memset flag and the
    last tanh's accum_out), so the real Ldweights need no waits.
"""

import sys

if "/opt/trn_rl_repo" not in sys.path:
    sys.path.insert(0, "/opt/trn_rl_repo")

import ml_dtypes
import numpy as np

B, S, H, R = 2, 128, 768, 4
KC = H // 128  # 6 h-chunks
N_CORES = 8
I_BLK = 32  # i's per A-tile macro block
N_BLKS = S // I_BLK
DRAIN_W = 4  # columns per PSUM drain batch (each column in its own bank)

BF16 = ml_dtypes.bfloat16

_PROGRAM_CACHE = {}
LAST_RESULTS = None  # BassKernelResults of the most recent run (for test.py)


def _build_program(split=True):
    import concourse.bass as bass
    import concourse.mybir as mybir
    from concourse.tile import TileContext

    f32 = mybir.dt.float32
    bf16 = mybir.dt.bfloat16

    nc = bass.Bass()

    # Inputs (per-core, host pre-transposed, bf16 except biases).
    # xt  : (128, 768)  [p, kc*128+i]  = x[b].T chunk layout
    # ws  : (128, 4608) [p, m*768+kc*128+j] = w_src_r.T slab layout
    # wt  : (128, 4608) same for w_tgt_r.T
    # bc  : (128, 6)    [p, m] = (b_src+b_tgt)[r*768+m*128+p]  (f32)
    # wo  : (128, 6)    [p, c] = w_out[c*128+p]
    xt_d = nc.dram_tensor("xt", [128, H], bf16, kind="ExternalInput")
    ws_d = nc.dram_tensor("ws", [128, KC * H], bf16, kind="ExternalInput")
    wt_d = nc.dram_tensor("wt", [128, KC * H], bf16, kind="ExternalInput")
    bc_d = nc.dram_tensor("bc", [128, KC], f32, kind="ExternalInput")
    wo_d = nc.dram_tensor("wo", [128, KC], bf16, kind="ExternalInput")
    out_d = nc.dram_tensor("outT", [S * S // 512, 512], f32, kind="ExternalOutput")

    Tanh = mybir.ActivationFunctionType.Tanh

    with TileContext(nc) as tc:
        with (
            tc.tile_pool(name="const", bufs=1) as const_pool,
            tc.tile_pool(name="wpool", bufs=1) as w_pool,
            tc.tile_pool(name="apool", bufs=2) as a_pool,
        ):
            x_t = const_pool.tile([128, H], bf16, tag="xt")
            bc_t = const_pool.tile([128, KC], f32, tag="bc")
            wo_t = const_pool.tile([128, KC], bf16, tag="wo")
            bc_v = const_pool.tile([128, KC], f32, tag="bcv")
            out_sb = const_pool.tile([1, S * S], f32, tag="osb")
            nc.sync.dma_start(out=x_t, in_=xt_d[:, :])
            nc.sync.dma_start(out=bc_t, in_=bc_d[:, :])
            nc.sync.dma_start(out=wo_t, in_=wo_d[:, :])
            # DVE-local copy of the bias so the tanh's only cross-engine
            # dep proc is DVE.
            nc.vector.tensor_copy(bc_v, bc_t)

            s_T = [const_pool.tile([128, 128], bf16, tag=f"s{m}", name=f"s_T{m}") for m in range(KC)]
            t_T = [const_pool.tile([128, 128], bf16, tag=f"t{m}", name=f"t_T{m}") for m in range(KC)]

            # ---- projections: s_T[m][h_local, i], t_T[m][h_local, j] ----
            with tc.tile_pool(name="psproj", bufs=2, space="PSUM") as ps_proj:
                for m in range(KC):
                    for side in ("s", "t"):
                        wm = w_pool.tile([128, H], bf16, tag=f"w{side}{m}", name=f"w{side}{m}")
                        src = ws_d if side == "s" else wt_d
                        nc.sync.dma_start(out=wm, in_=src[:, m * H : (m + 1) * H])
                        ps = ps_proj.tile([128, 128], f32, tag="pp", name=f"pp_{side}{m}")
                        for kc in range(KC):
                            nc.tensor.matmul(
                                ps,
                                wm[:, kc * 128 : (kc + 1) * 128],
                                x_t[:, kc * 128 : (kc + 1) * 128],
                                start=(kc == 0),
                                stop=(kc == KC - 1),
                            )
                        dst = s_T[m] if side == "s" else t_T[m]
                        nc.vector.tensor_copy(dst, ps)

            # ---- pairwise tanh + weighted reduction ----
            # Per (block, chunk): one fat broadcast tensor_add builds the
            # (128, I_BLK, 128) tanh-argument tile, one in-place Tanh (with
            # the combined per-partition bias), then the reduction streams
            # the tanh tile as the MOVING matmul operand (N=512) against the
            # stationary w_out chunk column, accumulating (1, 512) rows of
            # logits (pair-major [i, j]) in PSUM.
            with tc.tile_pool(name="psout", bufs=1, space="PSUM") as ps_out:
                for n in range(N_BLKS):
                    a_tiles = []
                    for c in range(KC):
                        a = a_pool.tile([128, I_BLK, 128], bf16, tag=f"a{c}", name=f"a{n}_{c}")
                        sblk = s_T[c][:, n * I_BLK : (n + 1) * I_BLK]
                        nc.vector.tensor_add(
                            a[:, :, :],
                            sblk.unsqueeze(2).broadcast_to((128, I_BLK, 128)),
                            t_T[c].unsqueeze(1).broadcast_to((128, I_BLK, 128)),
                        )
                        nc.scalar.activation(
                            a[:, :, :], a[:, :, :], Tanh,
                            bias=bc_v[:, c : c + 1], scale=1.0,
                        )
                        a_tiles.append(a)
                    # chunk-major reduction: 8 (1,512) PSUM rows (one bank
                    # each) accumulate across the c loop, so PE consumes each
                    # tanh tile as soon as it is ready.
                    pss = [
                        ps_out.tile([1, 512], f32, tag=f"lp{g}", name=f"lp{n}_{g}")
                        for g in range(I_BLK // 4)
                    ]
                    for c in range(KC):
                        for g in range(I_BLK // 4):
                            nc.tensor.matmul(
                                pss[g],
                                wo_t[:, c : c + 1],
                                a_tiles[c][:, g * 4 : (g + 1) * 4, :],
                                start=(c == 0),
                                stop=(c == KC - 1),
                            )
                    for g in range(I_BLK // 4):
                        row = n * (I_BLK // 4) + g
                        dst = out_sb[0:1, row * 512 : (row + 1) * 512]
                        if g % 2 == 0:
                            nc.vector.tensor_copy(dst, pss[g])
                        else:
                            nc.scalar.copy(dst, pss[g])
            nc.sync.dma_start(out=out_d[:, :], in_=out_sb)

    if split:
        _split_multi_waits(nc, mybir)
    return nc


def _split_multi_waits(nc, mybir):
    """This walrus build allows at most ONE sync-wait per instruction.
    Legalize by hoisting all but one wait onto same-engine NoOps placed
    immediately before the offending instruction (the engine executes its
    queue in order, so waiting on the NoOps first is equivalent)."""
    k = 0
    for func in nc.m.functions:
        for blk in func.blocks:
            insts = list(blk.instructions)
            out = []
            changed = False
            for inst in insts:
                si = inst.sync_info
                waits = list(si.on_wait) if si is not None and si.on_wait else []
                if len(waits) > 1:
                    changed = True
                    for w in waits[:-1]:
                        nop = mybir.InstNoOp(
                            name=f"WSPLIT-{k}",
                            engine=inst.engine,
                            sync_info=mybir.SyncInfo(on_wait=[w], on_update=[]),
                            ins=[],
                            outs=[],
                        )
                        k += 1
                        out.append(nop)
                    si.on_wait = [waits[-1]]
                out.append(inst)
            if changed:
                blk.instructions = out


def _prep_inputs(input_hidden_state, w_src, b_src, w_tgt, b_tgt, w_out):
    """Build the 8 per-core input dicts (host-side transpose/cast)."""
    x = np.asarray(input_hidden_state, dtype=np.float32)
    w_src = np.asarray(w_src, dtype=np.float32)
    w_tgt = np.asarray(w_tgt, dtype=np.float32)
    b_sum = np.asarray(b_src, dtype=np.float32) + np.asarray(b_tgt, dtype=np.float32)
    w_out = np.asarray(w_out, dtype=np.float32)

    wo_tile = np.ascontiguousarray(w_out.reshape(KC, 128).T).astype(BF16)

    in_maps = []
    for core in range(N_CORES):
        b, r = divmod(core, R)
        # xT chunk layout: xt[p, kc*128+i] = x[b][i, kc*128+p]
        xT = x[b].T  # (H, S)
        xt = np.ascontiguousarray(
            xT.reshape(KC, 128, S).transpose(1, 0, 2).reshape(128, H)
        ).astype(BF16)

        # ws[p, m*768 + kc*128 + j] = wT[kc*128+p, m*128+j],  wT = w_r.T
        wT_s = w_src[r * H : (r + 1) * H, :].T.reshape(KC, 128, KC, 128)
        ws = np.ascontiguousarray(
            wT_s.transpose(1, 2, 0, 3).reshape(128, KC * H)
        ).astype(BF16)
        wT_t = w_tgt[r * H : (r + 1) * H, :].T.reshape(KC, 128, KC, 128)
        wt = np.ascontiguousarray(
            wT_t.transpose(1, 2, 0, 3).reshape(128, KC * H)
        ).astype(BF16)

        bc = np.ascontiguousarray(
            b_sum[r * H : (r + 1) * H].reshape(KC, 128).T
        ).astype(np.float32)

        in_maps.append({"xt": xt, "ws": ws, "wt": wt, "bc": bc, "wo": wo_tile})
    return in_maps


def kernel(input_hidden_state, w_src, b_src, w_tgt, b_tgt, w_out):
    global LAST_RESULTS
    from concourse.bass_utils import run_bass_kernel_spmd

    if "prog" not in _PROGRAM_CACHE:
        _PROGRAM_CACHE["prog"] = _build_program()
    nc = _PROGRAM_CACHE["prog"]

    in_maps = _prep_inputs(
        input_hidden_state, w_src, b_src, w_tgt, b_tgt, w_out
    )
    res = run_bass_kernel_spmd(nc, in_maps, core_ids=list(range(N_CORES)))
    LAST_RESULTS = res

    out = np.empty((B, R, S, S), dtype=np.float32)
    for core in range(N_CORES):
        b, r = divmod(core, R)
        out[b, r] = np.asarray(res.results[core]["outT"], dtype=np.float32).reshape(S, S)
    return out
